# revision 10
# baseline (speedup 1.0000x reference)
"""BertAttention (B=8, S=1024, H=1024, 16 heads) on 8 TRN2 NeuronCores.

Strategy: data-parallel over batch -- core b computes batch element b
end-to-end (QKV proj, attention, output proj, residual, LayerNorm).
No collectives needed.

fp8 version: all big matmuls run in float8e4 (e4m3) with DoubleRow perf
mode (K=256 contraction per instruction, 2x PE throughput):
  - QKV projections: x, W in fp8; contraction over H = 4 DoubleRow steps.
  - Scores: q/k kept in bf16 (scores are PSUM-write-port-bound, dtype
    doesn't change cycles; bf16 improves accuracy for free).
  - PV: exp outputs written as fp8; DoubleRow pairs k-tiles; softmax
    denominators via a ones-column folded into V (row 64 of each head's
    65-wide V block).
  - Output projection: ctx in fp8, DoubleRow over ctx-tile pairs.
Engine budget per core: PE ~328k cycles (~137us), ACT exp ~133us
(overlapped), DVE does all PSUM->SBUF converts (GPSIMD cannot touch
PSUM), Pool does broadcasts + LN apply.

Emission order pipelines K[0],Q[0] first so ACT exp starts ~7us in, and
V-projection tiles are woven between per-head score blocks to avoid ACT
bubbles (scores PSUM double-buffering caps ACT run-ahead at 2 tiles).
"""

import sys

sys.path.insert(0, "/opt/trn_rl_repo")

import numpy as np

B, S, H = 8, 1024, 1024
NH, HD = 16, 64
LN_EPS = 1e-12
N_CORES = 8

MM_DTYPE = "fp8"  # "fp8" | "f32r"

_compiled = {}

NT = 8    # 128-row tiles per 1024 dim
NCH = 2   # 512-col chunks per 1024 dim
CH = 512
NKP = 4   # k-tile pairs (DoubleRow)


def _build_fp8(use_gb=True, n_reps=1):
    import concourse.tile as tile
    from concourse import bacc, mybir

    F32 = mybir.dt.float32
    FP8 = mybir.dt.float8e4
    BF16 = mybir.dt.bfloat16
    AF = mybir.ActivationFunctionType
    ALU = mybir.AluOpType
    DR = mybir.MatmulPerfMode.DoubleRow

    nc = bacc.Bacc("TRN2", target_bir_lowering=False)

    # swizzled layouts: t[p, kt*1024 + j] = M[kt*128 + p, j]
    xt_d = nc.dram_tensor("xt", [128, 8 * S], FP8, kind="ExternalInput")
    wq_d = nc.dram_tensor("wq", [128, 8 * H], FP8, kind="ExternalInput")
    wk_d = nc.dram_tensor("wk", [128, 8 * H], FP8, kind="ExternalInput")
    wv_d = nc.dram_tensor("wv", [128, 8 * H], FP8, kind="ExternalInput")
    wo_d = nc.dram_tensor("wo", [128, 8 * H], FP8, kind="ExternalInput")
    xr_d = nc.dram_tensor("xr", [S, H], F32, kind="ExternalInput")
    bq_d = nc.dram_tensor("bq", [128, 8], F32, kind="ExternalInput")
    bk_d = nc.dram_tensor("bk", [128, 8], F32, kind="ExternalInput")
    bv_d = nc.dram_tensor("bv", [1, H], F32, kind="ExternalInput")
    mask_d = nc.dram_tensor("mask", [128, 8], F32, kind="ExternalInput")
    gamma_d = nc.dram_tensor("gamma", [1, H], F32, kind="ExternalInput")
    beta_d = nc.dram_tensor("beta", [1, H], F32, kind="ExternalInput")
    out_d = nc.dram_tensor("out", [S, H], F32, kind="ExternalOutput")

    with tile.TileContext(nc) as tc:
      for _rep in range(n_reps):
        with (
            tc.tile_pool(name="consts", bufs=1) as cp,
            tc.tile_pool(name="wb", bufs=1) as wb,          # x + weights fp8
            tc.tile_pool(name="qk", bufs=1) as qkp,         # q/k bf16 tiles
            tc.tile_pool(name="vt", bufs=1) as vtp,         # v fp8 pair tiles
            tc.tile_pool(name="ctx", bufs=1) as ctxp_pool,  # ctx fp8 pair tiles
            tc.tile_pool(name="ep", bufs=8) as ep,          # exp fp8 pair tiles
            tc.tile_pool(name="rp", bufs=4) as rp,          # recip rows
            tc.tile_pool(name="rbp", bufs=4) as rbp,        # bcast recip
            tc.tile_pool(name="xrp", bufs=1) as xrp,        # residual tiles
            tc.tile_pool(name="ob", bufs=3) as obp,         # out f32 staging
            tc.tile_pool(name="st", bufs=4) as stp,         # LN stats
            tc.tile_pool(name="pp", bufs=2, space="PSUM") as pp,     # 2 banks
            tc.tile_pool(name="scps", bufs=2, space="PSUM") as scps, # 4 banks
            tc.tile_pool(name="cxps", bufs=2, space="PSUM") as cxps, # 2 banks
        ):
            # ---------------- DMAs: inputs ----------------
            xt8 = wb.tile([128, 8 * S], FP8, name="xt8")
            wq8 = wb.tile([128, 8 * H], FP8, name="wq8")
            wk8 = wb.tile([128, 8 * H], FP8, name="wk8")
            wv8 = wb.tile([128, 8 * H], FP8, name="wv8")
            wo8 = wb.tile([128, 8 * H], FP8, name="wo8")
            nc.sync.dma_start(out=xt8, in_=xt_d[:])
            nc.scalar.dma_start(out=wk8, in_=wk_d[:])
            nc.gpsimd.dma_start(out=wq8, in_=wq_d[:])
            nc.sync.dma_start(out=wv8, in_=wv_d[:])

            bq_sb = cp.tile([128, 8], F32)
            bk_sb = cp.tile([128, 8], F32)
            mask_sb = cp.tile([128, 8], F32)
            nc.sync.dma_start(out=bq_sb, in_=bq_d[:])
            nc.sync.dma_start(out=bk_sb, in_=bk_d[:])
            nc.sync.dma_start(out=mask_sb, in_=mask_d[:])
            bv_row = cp.tile([1, H], F32)
            gamma_row = cp.tile([1, H], F32)
            beta_row = cp.tile([1, H], F32)
            nc.sync.dma_start(out=bv_row, in_=bv_d[:])
            nc.sync.dma_start(out=gamma_row, in_=gamma_d[:])
            nc.sync.dma_start(out=beta_row, in_=beta_d[:])
            bv_sb = cp.tile([128, H], F32)
            nc.gpsimd.partition_broadcast(bv_sb[:], bv_row[:])
            if use_gb:
                gamma_sb = cp.tile([128, H], F32)
                beta_sb = cp.tile([128, H], F32)
                nc.gpsimd.partition_broadcast(gamma_sb[:], gamma_row[:])
                nc.gpsimd.partition_broadcast(beta_sb[:], beta_row[:])
            eps_sb = cp.tile([128, 1], F32)
            nc.vector.memset(eps_sb[:], LN_EPS)

            # q/k bf16 [dm 128, s 1024] per head-pair tile
            qt = [qkp.tile([128, S], BF16, name=f"qt{m}") for m in range(NT)]
            kt = [qkp.tile([128, S], BF16, name=f"kt{m}") for m in range(NT)]
            # v fp8 pair tiles: vp[j][p, kk, g*65+d]; s = (2j+kk)*128+p
            vp = [vtp.tile([128, 2 * NH * 65], FP8, name=f"vp{j}")
                  for j in range(NKP)]
            # ctx fp8 pair tiles: ctxp[j][p, tt, q]; c = (2j+tt)*128+p
            ctx = [ctxp_pool.tile([128, 2 * S], FP8, name=f"ctx{j}")
                   for j in range(NKP)]
            # ones columns for softmax denominators
            for j in range(NKP):
                v4 = vp[j][:].rearrange("p (i g e) -> p i g e", i=2, e=65)
                nc.gpsimd.memset(v4[:, :, :, 64:65], 1.0)

            def proj_qk(m, w8, b_sb, dst):
                # out [m-block 128, s]: lhsT = w slice, rhs = xt slices
                w3 = w8[:].rearrange("p (t j) -> p t j", t=8)
                x3 = xt8[:].rearrange("p (t j) -> p t j", t=8)
                for n in range(NCH):
                    ps = pp.tile([128, CH], F32, tag="pp", name="pp_t")
                    for r in range(4):
                        nc.tensor.matmul(
                            ps[:],
                            lhsT=w3[:, 2 * r:2 * r + 2, m * 128:(m + 1) * 128],
                            rhs=x3[:, 2 * r:2 * r + 2, n * CH:(n + 1) * CH],
                            start=(r == 0), stop=(r == 3), perf_mode=DR,
                        )
                    nc.vector.tensor_scalar_add(
                        dst[m][:, n * CH:(n + 1) * CH], ps[:], b_sb[:, m:m + 1]
                    )

            def proj_v(mk):
                # out [s-block 128, dv]: lhsT = xt slice, rhs = wv slices
                w3 = wv8[:].rearrange("p (t j) -> p t j", t=8)
                x3 = xt8[:].rearrange("p (t j) -> p t j", t=8)
                j, kk = mk // 2, mk % 2
                v4 = vp[j][:].rearrange("p (i g e) -> p i g e", i=2, e=65)
                for n in range(NCH):
                    ps = pp.tile([128, CH], F32, tag="pp", name="pp_t")
                    for r in range(4):
                        nc.tensor.matmul(
                            ps[:],
                            lhsT=x3[:, 2 * r:2 * r + 2, mk * 128:(mk + 1) * 128],
                            rhs=w3[:, 2 * r:2 * r + 2, n * CH:(n + 1) * CH],
                            start=(r == 0), stop=(r == 3), perf_mode=DR,
                        )
                    nc.vector.tensor_add(
                        v4[:, kk, n * 8:(n + 1) * 8, 0:64],
                        ps[:].rearrange("p (g e) -> p g e", e=64),
                        bv_sb[:, n * CH:(n + 1) * CH]
                        .rearrange("p (g e) -> p g e", e=64),
                    )

            import os
            ablate = os.environ.get("ABLATE", "")

            def head_block(g, cx, fillers=()):
                # scores + exp + PV for head g; cx = [cx_n0, cx_n1] accumulators
                # fillers: per-kp callables emitted between exp and PV, used to
                # weave V/QK projections into the head without starving ACT.
                t, hh = g // 2, g % 2
                p0 = hh * 64
                for kp in range(NKP):
                    e_t = ep.tile([128, 2 * S], FP8, tag="e", name="e_t")
                    e3 = e_t[:].rearrange("p (i q) -> p i q", i=2)
                    for kk in range(2):
                        k = 2 * kp + kk
                        sc = scps.tile([128, S], F32, tag="sc", name="sc_t")
                        for n in range(NCH):
                            if ablate == "sc" and n == 1:
                                continue
                            nc.tensor.matmul(
                                sc[:, n * CH:(n + 1) * CH],
                                lhsT=kt[t][p0:p0 + 64, k * 128:(k + 1) * 128],
                                rhs=qt[t][p0:p0 + 64, n * CH:(n + 1) * CH],
                                start=True, stop=True,
                            )
                        if ablate == "exp" and kk == 1:
                            continue
                        nc.scalar.activation(
                            e3[:, kk, :], sc[:], AF.Exp,
                            bias=mask_sb[:, k:k + 1], scale=0.125,
                        )
                    if kp < len(fillers) and fillers[kp] is not None:
                        fillers[kp]()
                    v4 = vp[kp][:].rearrange("p (i m) -> p i m", i=2)
                    for n in range(NCH):
                        nc.tensor.matmul(
                            cx[n][:],
                            lhsT=v4[:, :, g * 65:(g + 1) * 65],
                            rhs=e3[:, :, n * CH:(n + 1) * CH],
                            start=(kp == 0), stop=(kp == NKP - 1),
                            perf_mode=DR,
                        )

            def normalize(g, cx):
                t, hh = g // 2, g % 2
                j, tt = t // 2, t % 2
                c3 = ctx[j][:].rearrange("p (i q) -> p i q", i=2)
                for n in range(NCH):
                    recip = rp.tile([1, CH], F32, tag="recip", name="recip_t")
                    nc.vector.reciprocal(recip[:], cx[n][64:65, :])
                    rb = rbp.tile([64, CH], F32, tag="rb", name="rb_t")
                    nc.gpsimd.partition_broadcast(rb[:], recip[:])
                    nc.vector.tensor_mul(
                        c3[hh * 64:hh * 64 + 64, tt, n * CH:(n + 1) * CH],
                        cx[n][0:64, :], rb[:],
                    )

            # ---------------- pipelined emission ----------------
            # Emission order IS the per-engine execution order and the
            # semantic (trace) order: every tile must be written before a
            # later-emitted reader. K[0]/Q[0] first so ACT exp starts early;
            # V-proj pairs are woven INSIDE head 0 (each before the PV(kp)
            # that consumes them), K/Q tile production into later heads.
            proj_qk(0, wk8, bk_sb, kt)
            proj_qk(0, wq8, bq_sb, qt)

            cx_of = {}

            def attn_head(g, fillers=()):
                cx = [cxps.tile([65, CH], F32, tag="cx", name="cx_t")
                      for _ in range(NCH)]
                cx_of[g] = cx
                head_block(g, cx, fillers)
                if g > 0:
                    normalize(g - 1, cx_of.pop(g - 1))

            attn_head(0, fillers=(
                lambda: (proj_v(0), proj_v(1)),
                lambda: (proj_v(2), proj_v(3)),
                lambda: (proj_v(4), proj_v(5)),
                lambda: (proj_v(6), proj_v(7)),
            ))
            attn_head(1, fillers=(
                lambda: proj_qk(1, wk8, bk_sb, kt),
                lambda: proj_qk(1, wq8, bq_sb, qt),
            ))
            attn_head(2, fillers=(
                lambda: proj_qk(2, wk8, bk_sb, kt),
            ))
            attn_head(3, fillers=(
                lambda: proj_qk(2, wq8, bq_sb, qt),
            ))

            def dma_prefetch():
                # wo + xr prefetch now that input DMA queues are free
                nc.scalar.dma_start(out=wo8, in_=wo_d[:])
                for mq in range(NT):
                    xr_t = xrp.tile([128, H], F32, name=f"xr{mq}")
                    (nc.sync if mq % 2 == 0 else nc.gpsimd).dma_start(
                        out=xr_t, in_=xr_d[mq * 128:(mq + 1) * 128, :]
                    )
                    xr_tiles.append(xr_t)

            xr_tiles = []
            for t in range(2, NT):
                g0 = 2 * t
                f0 = [lambda tt=t: proj_qk(tt + 1, wk8, bk_sb, kt)] \
                    if t + 1 < NT else []
                f1 = [lambda tt=t: proj_qk(tt + 1, wq8, bq_sb, qt)] \
                    if t + 1 < NT else []
                if t == 2:
                    f0.append(dma_prefetch)
                attn_head(g0, fillers=tuple(f0))
                attn_head(g0 + 1, fillers=tuple(f1))
            normalize(15, cx_of.pop(15))

            # ---------------- output proj + residual + LayerNorm ----------------
            wo3 = wo8[:].rearrange("p (t j) -> p t j", t=8)
            for mq in range(NT):
                o_t = obp.tile([128, H], F32, tag="ob", name="ob_t")
                for n in range(NCH):
                    ps = pp.tile([128, CH], F32, tag="pp", name="pp_t")
                    for j in range(NKP):
                        c3 = ctx[j][:].rearrange("p (i q) -> p i q", i=2)
                        nc.tensor.matmul(
                            ps[:],
                            lhsT=c3[:, :, mq * 128:(mq + 1) * 128],
                            rhs=wo3[:, 2 * j:2 * j + 2, n * CH:(n + 1) * CH],
                            start=(j == 0), stop=(j == NKP - 1), perf_mode=DR,
                        )
                    nc.vector.tensor_add(
                        o_t[:, n * CH:(n + 1) * CH], ps[:],
                        xr_tiles[mq][:, n * CH:(n + 1) * CH],
                    )
                stats = stp.tile([128, 2, 6], F32, tag="stats", name="stats_t")
                for sg in range(2):
                    nc.vector.bn_stats(
                        stats[:, sg, :], o_t[:, sg * CH:(sg + 1) * CH]
                    )
                mv = stp.tile([128, 2], F32, tag="mv", name="mv_t")
                nc.vector.bn_aggr(mv[:], stats[:])
                mu = mv[:, 0:1]
                var = mv[:, 1:2]
                std = stp.tile([128, 1], F32, tag="std", name="std_t")
                nc.scalar.activation(std[:], var[:], AF.Sqrt, bias=eps_sb[:])
                rstd = stp.tile([128, 1], F32, tag="rstd", name="rstd_t")
                nc.vector.reciprocal(rstd[:], std[:])
                # (x - mu) * rstd = rstd*x + (-mu*rstd)
                nmur = stp.tile([128, 1], F32, tag="nmur", name="nmur_t")
                nc.vector.tensor_scalar(
                    out=nmur[:], in0=mu, scalar1=rstd[:], scalar2=-1.0,
                    op0=ALU.mult, op1=ALU.mult,
                )
                of_t = obp.tile([128, H], F32, tag="ob", name="of_t")
                nc.gpsimd.tensor_scalar(
                    out=of_t[:], in0=o_t[:], scalar1=rstd[:], scalar2=nmur[:],
                    op0=ALU.mult, op1=ALU.add,
                )
                if use_gb:
                    nc.gpsimd.tensor_mul(of_t[:], of_t[:], gamma_sb[:])
                    nc.gpsimd.tensor_add(of_t[:], of_t[:], beta_sb[:])
                (nc.gpsimd if mq % 2 == 0 else nc.sync).dma_start(
                    out=out_d[mq * 128:(mq + 1) * 128, :], in_=of_t
                )

    nc.compile()
    return nc


def _host_prep_fp8(hidden_states, attention_mask, Wq, bq, Wk, bk, Wv, bv,
                   Wo, bo, ln_gamma, ln_beta):
    import ml_dtypes

    f32 = np.float32
    fp8 = ml_dtypes.float8_e4m3
    hs = np.ascontiguousarray(hidden_states, dtype=f32)

    def swz(mat):
        # [1024, J] -> [128, 8*J] with t[p, kt*J + j] = mat[kt*128 + p, j]
        m = np.asarray(mat, f32).reshape(8, 128, -1).transpose(1, 0, 2)
        return np.ascontiguousarray(m.reshape(128, -1).astype(fp8))

    wq8 = swz(np.asarray(Wq, f32).T)
    wk8 = swz(np.asarray(Wk, f32).T)
    wv8 = swz(np.asarray(Wv, f32).T)
    wo8 = swz(np.asarray(Wo, f32).T)
    bq_r = np.ascontiguousarray(np.asarray(bq, f32).reshape(8, 128).T)
    bk_r = np.ascontiguousarray(np.asarray(bk, f32).reshape(8, 128).T)
    bv_r = np.ascontiguousarray(np.asarray(bv, f32).reshape(1, H))
    gamma_r = np.ascontiguousarray(np.asarray(ln_gamma, f32).reshape(1, H))
    beta_r = np.ascontiguousarray(np.asarray(ln_beta, f32).reshape(1, H))
    bo_r = np.asarray(bo, f32)
    mask = np.asarray(attention_mask, f32).reshape(B, S)

    in_maps = []
    for b in range(B):
        xt8 = swz(hs[b].T)
        xr = np.ascontiguousarray(hs[b] + bo_r[None, :])
        mask_r = np.ascontiguousarray(mask[b].reshape(8, 128).T)
        in_maps.append({
            "xt": xt8, "xr": xr,
            "wq": wq8, "wk": wk8, "wv": wv8, "wo": wo8,
            "bq": bq_r, "bk": bk_r, "bv": bv_r,
            "mask": mask_r, "gamma": gamma_r, "beta": beta_r,
        })
    return in_maps


def _host_prep(mm_dtype, **kw):
    assert mm_dtype == "fp8"
    return _host_prep_fp8(**kw)


def get_nc(mm_dtype=MM_DTYPE, n_reps=1, use_gb=True):
    key = (mm_dtype, n_reps, use_gb)
    if key not in _compiled:
        assert mm_dtype == "fp8"
        _compiled[key] = _build_fp8(use_gb, n_reps)
    return _compiled[key]


def kernel(hidden_states, attention_mask, Wq, bq, Wk, bk, Wv, bv, Wo, bo,
           ln_gamma, ln_beta):
    from concourse.bass_utils import run_bass_kernel_spmd

    use_gb = not (
        np.all(np.asarray(ln_gamma) == 1.0) and np.all(np.asarray(ln_beta) == 0.0)
    )
    nc = get_nc(MM_DTYPE, use_gb=use_gb)
    in_maps = _host_prep(MM_DTYPE, hidden_states=hidden_states,
                         attention_mask=attention_mask, Wq=Wq, bq=bq, Wk=Wk,
                         bk=bk, Wv=Wv, bv=bv, Wo=Wo, bo=bo,
                         ln_gamma=ln_gamma, ln_beta=ln_beta)
    res = run_bass_kernel_spmd(nc, in_maps, list(range(N_CORES)))
    out = np.stack([np.asarray(res.results[i]["out"]) for i in range(N_CORES)])
    return out.astype(np.float32)


# revision 20
# speedup vs baseline: 1.6899x; 1.6899x over previous
"""BertAttention (B=8, S=1024, H=1024, 16 heads) on 8 TRN2 NeuronCores.

Strategy: data-parallel over batch -- core b computes batch element b
end-to-end (QKV proj, attention, output proj, residual, LayerNorm).
No collectives needed.

fp8 version: all big matmuls run in float8e4 (e4m3) with DoubleRow perf
mode (K=256 contraction per instruction, 2x PE throughput):
  - QKV projections: x, W in fp8; contraction over H = 4 DoubleRow steps.
  - Scores: q/k kept in bf16 (scores are PSUM-write-port-bound, dtype
    doesn't change cycles; bf16 improves accuracy for free).
  - PV: exp outputs written as fp8; DoubleRow pairs k-tiles; softmax
    denominators via a ones-column folded into V (row 64 of each head's
    65-wide V block).
  - Output projection: ctx in fp8, DoubleRow over ctx-tile pairs.
Engine budget per core: PE ~328k cycles (~137us), ACT exp ~133us
(overlapped), DVE does all PSUM->SBUF converts (GPSIMD cannot touch
PSUM), Pool does broadcasts + LN apply.

Emission order pipelines K[0],Q[0] first so ACT exp starts ~7us in, and
V-projection tiles are woven between per-head score blocks to avoid ACT
bubbles (scores PSUM double-buffering caps ACT run-ahead at 2 tiles).
"""

import sys

sys.path.insert(0, "/opt/trn_rl_repo")

import numpy as np

B, S, H = 8, 1024, 1024
NH, HD = 16, 64
LN_EPS = 1e-12
N_CORES = 8

MM_DTYPE = "fp8"  # "fp8" | "f32r"

_compiled = {}

NT = 8    # 128-row tiles per 1024 dim
NCH = 2   # 512-col chunks per 1024 dim
CH = 512
NKP = 4   # k-tile pairs (DoubleRow)


def _build_fp8(use_gb=True, n_reps=1):
    import concourse.tile as tile
    from concourse import bacc, mybir

    F32 = mybir.dt.float32
    FP8 = mybir.dt.float8e4
    BF16 = mybir.dt.bfloat16
    AF = mybir.ActivationFunctionType
    ALU = mybir.AluOpType
    DR = mybir.MatmulPerfMode.DoubleRow

    nc = bacc.Bacc("TRN2", target_bir_lowering=False)

    # swizzled layouts: t[p, kt*1024 + j] = M[kt*128 + p, j]
    xt_d = nc.dram_tensor("xt", [128, 8 * S], FP8, kind="ExternalInput")
    wq_d = nc.dram_tensor("wq", [128, 8 * H], FP8, kind="ExternalInput")
    wk_d = nc.dram_tensor("wk", [128, 8 * H], FP8, kind="ExternalInput")
    wv_d = nc.dram_tensor("wv", [128, 8 * H], FP8, kind="ExternalInput")
    wo_d = nc.dram_tensor("wo", [128, 8 * H], FP8, kind="ExternalInput")
    xr_d = nc.dram_tensor("xr", [S, H], F32, kind="ExternalInput")
    bq_d = nc.dram_tensor("bq", [128, 8], F32, kind="ExternalInput")
    bk_d = nc.dram_tensor("bk", [128, 8], F32, kind="ExternalInput")
    bv_d = nc.dram_tensor("bv", [1, H], F32, kind="ExternalInput")
    mask_d = nc.dram_tensor("mask", [128, 8], F32, kind="ExternalInput")
    gamma_d = nc.dram_tensor("gamma", [1, H], F32, kind="ExternalInput")
    beta_d = nc.dram_tensor("beta", [1, H], F32, kind="ExternalInput")
    out_d = nc.dram_tensor("out", [S, H], F32, kind="ExternalOutput")

    with tile.TileContext(nc) as tc:
      for _rep in range(n_reps):
        with (
            tc.tile_pool(name="consts", bufs=1) as cp,
            tc.tile_pool(name="wb", bufs=1) as wb,          # x + weights fp8
            tc.tile_pool(name="qk", bufs=1) as qkp,         # q/k bf16 tiles
            tc.tile_pool(name="vt", bufs=1) as vtp,         # v fp8 pair tiles
            tc.tile_pool(name="ctx", bufs=1) as ctxp_pool,  # ctx fp8 pair tiles
            tc.tile_pool(name="ep", bufs=8) as ep,          # exp fp8 pair tiles
            tc.tile_pool(name="rp", bufs=4) as rp,          # recip rows
            tc.tile_pool(name="rbp", bufs=4) as rbp,        # bcast recip
            tc.tile_pool(name="xrp", bufs=1) as xrp,        # residual tiles
            tc.tile_pool(name="ob", bufs=4) as obp,         # out f32 staging
            tc.tile_pool(name="st", bufs=4) as stp,         # LN stats
            tc.tile_pool(name="pp", bufs=2, space="PSUM") as pp,     # 2 banks
            tc.tile_pool(name="scps", bufs=2, space="PSUM") as scps, # 4 banks
            tc.tile_pool(name="cxps", bufs=2, space="PSUM") as cxps, # 2 banks
        ):
            # ---------------- DMAs: inputs ----------------
            xt8 = wb.tile([128, 8 * S], FP8, name="xt8")
            wq8 = wb.tile([128, 8 * H], FP8, name="wq8")
            wk8 = wb.tile([128, 8 * H], FP8, name="wk8")
            wv8 = wb.tile([128, 8 * H], FP8, name="wv8")
            wo8 = wb.tile([128, 8 * H], FP8, name="wo8")
            nc.sync.dma_start(out=xt8, in_=xt_d[:])
            nc.scalar.dma_start(out=wk8, in_=wk_d[:])
            nc.gpsimd.dma_start(out=wq8, in_=wq_d[:])
            nc.sync.dma_start(out=wv8, in_=wv_d[:])

            bq_sb = cp.tile([128, 8], F32)
            bk_sb = cp.tile([128, 8], F32)
            mask_sb = cp.tile([128, 8], F32)
            nc.sync.dma_start(out=bq_sb, in_=bq_d[:])
            nc.sync.dma_start(out=bk_sb, in_=bk_d[:])
            nc.sync.dma_start(out=mask_sb, in_=mask_d[:])
            bv_row = cp.tile([1, H], F32)
            gamma_row = cp.tile([1, H], F32)
            beta_row = cp.tile([1, H], F32)
            nc.sync.dma_start(out=bv_row, in_=bv_d[:])
            nc.sync.dma_start(out=gamma_row, in_=gamma_d[:])
            nc.sync.dma_start(out=beta_row, in_=beta_d[:])
            bv_sb = cp.tile([128, H], F32)
            nc.gpsimd.partition_broadcast(bv_sb[:], bv_row[:])
            if use_gb:
                gamma_sb = cp.tile([128, H], F32)
                beta_sb = cp.tile([128, H], F32)
                nc.gpsimd.partition_broadcast(gamma_sb[:], gamma_row[:])
                nc.gpsimd.partition_broadcast(beta_sb[:], beta_row[:])
            eps_sb = cp.tile([128, 1], F32)
            nc.vector.memset(eps_sb[:], LN_EPS)

            # q/k bf16 [dm 128, s 1024] per head-pair tile
            qt = [qkp.tile([128, S], BF16, name=f"qt{m}") for m in range(NT)]
            kt = [qkp.tile([128, S], BF16, name=f"kt{m}") for m in range(NT)]
            # v fp8 pair tiles: vp[j][p, kk, g*65+d]; s = (2j+kk)*128+p
            vp = [vtp.tile([128, 2 * NH * 65], FP8, name=f"vp{j}")
                  for j in range(NKP)]
            # ctx fp8 pair tiles: ctxp[j][p, tt, q]; c = (2j+tt)*128+p
            ctx = [ctxp_pool.tile([128, 2 * S], FP8, name=f"ctx{j}")
                   for j in range(NKP)]
            # ones columns for softmax denominators
            for j in range(NKP):
                v4 = vp[j][:].rearrange("p (i g e) -> p i g e", i=2, e=65)
                nc.gpsimd.memset(v4[:, :, :, 64:65], 1.0)

            def proj_qk(m, w8, b_sb, dst):
                # out [m-block 128, s]: lhsT = w slice, rhs = xt slices
                w3 = w8[:].rearrange("p (t j) -> p t j", t=8)
                x3 = xt8[:].rearrange("p (t j) -> p t j", t=8)
                for n in range(NCH):
                    ps = pp.tile([128, CH], F32, tag="pp", name="pp_t")
                    for r in range(4):
                        nc.tensor.matmul(
                            ps[:],
                            lhsT=w3[:, 2 * r:2 * r + 2, m * 128:(m + 1) * 128],
                            rhs=x3[:, 2 * r:2 * r + 2, n * CH:(n + 1) * CH],
                            start=(r == 0), stop=(r == 3), perf_mode=DR,
                        )
                    nc.vector.tensor_scalar_add(
                        dst[m][:, n * CH:(n + 1) * CH], ps[:], b_sb[:, m:m + 1]
                    )

            def proj_v(mk):
                # out [s-block 128, dv]: lhsT = xt slice, rhs = wv slices
                w3 = wv8[:].rearrange("p (t j) -> p t j", t=8)
                x3 = xt8[:].rearrange("p (t j) -> p t j", t=8)
                j, kk = mk // 2, mk % 2
                v4 = vp[j][:].rearrange("p (i g e) -> p i g e", i=2, e=65)
                for n in range(NCH):
                    ps = pp.tile([128, CH], F32, tag="pp", name="pp_t")
                    for r in range(4):
                        nc.tensor.matmul(
                            ps[:],
                            lhsT=x3[:, 2 * r:2 * r + 2, mk * 128:(mk + 1) * 128],
                            rhs=w3[:, 2 * r:2 * r + 2, n * CH:(n + 1) * CH],
                            start=(r == 0), stop=(r == 3), perf_mode=DR,
                        )
                    nc.vector.tensor_add(
                        v4[:, kk, n * 8:(n + 1) * 8, 0:64],
                        ps[:].rearrange("p (g e) -> p g e", e=64),
                        bv_sb[:, n * CH:(n + 1) * CH]
                        .rearrange("p (g e) -> p g e", e=64),
                    )

            import os
            ablate = os.environ.get("ABLATE", "")

            def head_block(g, cx, fillers=()):
                # scores + exp + PV for head g; cx = [cx_n0, cx_n1] accumulators
                # fillers: per-kp callables emitted between exp and PV, used to
                # weave V/QK projections into the head without starving ACT.
                t, hh = g // 2, g % 2
                p0 = hh * 64

                def pv(kp, e3):
                    v4 = vp[kp][:].rearrange("p (i m) -> p i m", i=2)
                    for n in range(NCH):
                        nc.tensor.matmul(
                            cx[n][:],
                            lhsT=v4[:, :, g * 65:(g + 1) * 65],
                            rhs=e3[:, :, n * CH:(n + 1) * CH],
                            start=(kp == 0), stop=(kp == NKP - 1),
                            perf_mode=DR,
                        )

                pend = None  # PV runs one kp group behind scores, so a PV
                for kp in range(NKP):  # stall never blocks score production
                    e_t = ep.tile([128, 2 * S], FP8, tag="e", name="e_t")
                    e3 = e_t[:].rearrange("p (i q) -> p i q", i=2)
                    for kk in range(2):
                        k = 2 * kp + kk
                        sc = scps.tile([128, S], F32, tag="sc", name="sc_t")
                        for n in range(NCH):
                            if ablate == "sc" and n == 1:
                                continue
                            nc.tensor.matmul(
                                sc[:, n * CH:(n + 1) * CH],
                                lhsT=kt[t][p0:p0 + 64, k * 128:(k + 1) * 128],
                                rhs=qt[t][p0:p0 + 64, n * CH:(n + 1) * CH],
                                start=True, stop=True,
                            )
                        if ablate == "exp" and kk == 1:
                            continue
                        nc.scalar.activation(
                            e3[:, kk, :], sc[:], AF.Exp,
                            bias=mask_sb[:, k:k + 1], scale=0.125,
                        )
                    if kp < len(fillers) and fillers[kp] is not None:
                        fillers[kp]()
                    if pend is not None:
                        pv(*pend)
                    pend = (kp, e3)
                pv(*pend)

            def normalize(g, cx):
                t, hh = g // 2, g % 2
                j, tt = t // 2, t % 2
                c3 = ctx[j][:].rearrange("p (i q) -> p i q", i=2)
                for n in range(NCH):
                    recip = rp.tile([1, CH], F32, tag="recip", name="recip_t")
                    nc.vector.reciprocal(recip[:], cx[n][64:65, :])
                    rb = rbp.tile([64, CH], F32, tag="rb", name="rb_t")
                    nc.gpsimd.partition_broadcast(rb[:], recip[:])
                    nc.vector.tensor_mul(
                        c3[hh * 64:hh * 64 + 64, tt, n * CH:(n + 1) * CH],
                        cx[n][0:64, :], rb[:],
                    )

            # ---------------- pipelined emission ----------------
            # Emission order IS the per-engine execution order and the
            # semantic (trace) order: every tile must be written before a
            # later-emitted reader. K[0]/Q[0] first so ACT exp starts early;
            # V-proj pairs are woven INSIDE head 0 (each before the PV(kp)
            # that consumes them), K/Q tile production into later heads.
            proj_qk(0, wk8, bk_sb, kt)
            proj_qk(0, wq8, bq_sb, qt)

            cx_of = {}

            def attn_head(g, fillers=()):
                if g > 0:
                    normalize(g - 1, cx_of.pop(g - 1))
                cx = [cxps.tile([65, CH], F32, tag="cx", name="cx_t")
                      for _ in range(NCH)]
                cx_of[g] = cx
                head_block(g, cx, fillers)

            attn_head(0, fillers=(
                lambda: (proj_v(0), proj_v(1)),
                lambda: (proj_v(2), proj_v(3)),
                lambda: (proj_v(4), proj_v(5)),
                lambda: (proj_v(6), proj_v(7)),
            ))
            attn_head(1, fillers=(
                lambda: proj_qk(1, wk8, bk_sb, kt),
                lambda: proj_qk(1, wq8, bq_sb, qt),
            ))
            attn_head(2, fillers=(
                lambda: proj_qk(2, wk8, bk_sb, kt),
            ))
            attn_head(3, fillers=(
                lambda: proj_qk(2, wq8, bq_sb, qt),
            ))

            def dma_prefetch():
                # wo + xr prefetch now that input DMA queues are free
                nc.scalar.dma_start(out=wo8, in_=wo_d[:])
                for mq in range(NT):
                    xr_t = xrp.tile([128, H], F32, name=f"xr{mq}")
                    (nc.sync if mq % 2 == 0 else nc.gpsimd).dma_start(
                        out=xr_t, in_=xr_d[mq * 128:(mq + 1) * 128, :]
                    )
                    xr_tiles.append(xr_t)

            xr_tiles = []
            for t in range(2, NT):
                g0 = 2 * t
                f0 = [lambda tt=t: proj_qk(tt + 1, wk8, bk_sb, kt)] \
                    if t + 1 < NT else []
                f1 = [lambda tt=t: proj_qk(tt + 1, wq8, bq_sb, qt)] \
                    if t + 1 < NT else []
                if t == 2:
                    f0.append(dma_prefetch)
                attn_head(g0, fillers=tuple(f0))
                attn_head(g0 + 1, fillers=tuple(f1))
            normalize(15, cx_of.pop(15))

            # ---------------- output proj + residual + LayerNorm ----------------
            # Post-sqrt tail (recip/nmur/apply/DMA) runs one mq behind, so
            # DVE's heavy fused-add never queues behind the ACT sqrt
            # round-trip (in-order DVE queue).
            wo3 = wo8[:].rearrange("p (t j) -> p t j", t=8)
            pend_ln = []

            def ln_tail(mq, o_t, mu, std):
                rstd = stp.tile([128, 1], F32, tag="rstd", name="rstd_t")
                nc.vector.reciprocal(rstd[:], std[:])
                # (x - mu) * rstd = rstd*x + (-mu*rstd)
                nmur = stp.tile([128, 1], F32, tag="nmur", name="nmur_t")
                nc.vector.tensor_scalar(
                    out=nmur[:], in0=mu[:], scalar1=rstd[:], scalar2=-1.0,
                    op0=ALU.mult, op1=ALU.mult,
                )
                of_t = obp.tile([128, H], F32, tag="ob", name="of_t")
                nc.gpsimd.tensor_scalar(
                    out=of_t[:], in0=o_t[:], scalar1=rstd[:], scalar2=nmur[:],
                    op0=ALU.mult, op1=ALU.add,
                )
                if use_gb:
                    nc.gpsimd.tensor_mul(of_t[:], of_t[:], gamma_sb[:])
                    nc.gpsimd.tensor_add(of_t[:], of_t[:], beta_sb[:])
                (nc.sync if mq % 2 == 0 else nc.scalar).dma_start(
                    out=out_d[mq * 128:(mq + 1) * 128, :], in_=of_t
                )

            for mq in range(NT):
                o_t = obp.tile([128, H], F32, tag="ob", name="ob_t")
                # O accumulator reuses the (now idle) scores psum pool
                ps = scps.tile([128, S], F32, tag="sc", name="sc_t")
                for n in range(NCH):
                    for j in range(NKP):
                        c3 = ctx[j][:].rearrange("p (i q) -> p i q", i=2)
                        nc.tensor.matmul(
                            ps[:, n * CH:(n + 1) * CH],
                            lhsT=c3[:, :, mq * 128:(mq + 1) * 128],
                            rhs=wo3[:, 2 * j:2 * j + 2, n * CH:(n + 1) * CH],
                            start=(j == 0), stop=(j == NKP - 1), perf_mode=DR,
                        )
                nc.vector.tensor_add(o_t[:], ps[:], xr_tiles[mq][:])
                stats = stp.tile([128, 2, 6], F32, tag="stats", name="stats_t")
                for sg in range(2):
                    nc.vector.bn_stats(
                        stats[:, sg, :], o_t[:, sg * CH:(sg + 1) * CH]
                    )
                mv = stp.tile([128, 2], F32, tag="mv", name="mv_t")
                nc.vector.bn_aggr(mv[:], stats[:])
                mu = mv[:, 0:1]
                var = mv[:, 1:2]
                std = stp.tile([128, 1], F32, tag="std", name="std_t")
                nc.scalar.activation(std[:], var[:], AF.Sqrt, bias=eps_sb[:])
                if pend_ln:
                    ln_tail(*pend_ln.pop())
                pend_ln.append((mq, o_t, mu, std))
            ln_tail(*pend_ln.pop())

    nc.compile()
    return nc


def _host_prep_fp8(hidden_states, attention_mask, Wq, bq, Wk, bk, Wv, bv,
                   Wo, bo, ln_gamma, ln_beta):
    import ml_dtypes

    f32 = np.float32
    fp8 = ml_dtypes.float8_e4m3
    hs = np.ascontiguousarray(hidden_states, dtype=f32)

    def swz(mat):
        # [1024, J] -> [128, 8*J] with t[p, kt*J + j] = mat[kt*128 + p, j]
        m = np.asarray(mat, f32).reshape(8, 128, -1).transpose(1, 0, 2)
        return np.ascontiguousarray(m.reshape(128, -1).astype(fp8))

    wq8 = swz(np.asarray(Wq, f32).T)
    wk8 = swz(np.asarray(Wk, f32).T)
    wv8 = swz(np.asarray(Wv, f32).T)
    wo8 = swz(np.asarray(Wo, f32).T)
    bq_r = np.ascontiguousarray(np.asarray(bq, f32).reshape(8, 128).T)
    bk_r = np.ascontiguousarray(np.asarray(bk, f32).reshape(8, 128).T)
    bv_r = np.ascontiguousarray(np.asarray(bv, f32).reshape(1, H))
    gamma_r = np.ascontiguousarray(np.asarray(ln_gamma, f32).reshape(1, H))
    beta_r = np.ascontiguousarray(np.asarray(ln_beta, f32).reshape(1, H))
    bo_r = np.asarray(bo, f32)
    mask = np.asarray(attention_mask, f32).reshape(B, S)

    in_maps = []
    for b in range(B):
        xt8 = swz(hs[b].T)
        xr = np.ascontiguousarray(hs[b] + bo_r[None, :])
        mask_r = np.ascontiguousarray(mask[b].reshape(8, 128).T)
        in_maps.append({
            "xt": xt8, "xr": xr,
            "wq": wq8, "wk": wk8, "wv": wv8, "wo": wo8,
            "bq": bq_r, "bk": bk_r, "bv": bv_r,
            "mask": mask_r, "gamma": gamma_r, "beta": beta_r,
        })
    return in_maps


def _host_prep(mm_dtype, **kw):
    assert mm_dtype == "fp8"
    return _host_prep_fp8(**kw)


def get_nc(mm_dtype=MM_DTYPE, n_reps=1, use_gb=True):
    key = (mm_dtype, n_reps, use_gb)
    if key not in _compiled:
        assert mm_dtype == "fp8"
        _compiled[key] = _build_fp8(use_gb, n_reps)
    return _compiled[key]


def kernel(hidden_states, attention_mask, Wq, bq, Wk, bk, Wv, bv, Wo, bo,
           ln_gamma, ln_beta):
    from concourse.bass_utils import run_bass_kernel_spmd

    use_gb = not (
        np.all(np.asarray(ln_gamma) == 1.0) and np.all(np.asarray(ln_beta) == 0.0)
    )
    nc = get_nc(MM_DTYPE, use_gb=use_gb)
    in_maps = _host_prep(MM_DTYPE, hidden_states=hidden_states,
                         attention_mask=attention_mask, Wq=Wq, bq=bq, Wk=Wk,
                         bk=bk, Wv=Wv, bv=bv, Wo=Wo, bo=bo,
                         ln_gamma=ln_gamma, ln_beta=ln_beta)
    res = run_bass_kernel_spmd(nc, in_maps, list(range(N_CORES)))
    out = np.stack([np.asarray(res.results[i]["out"]) for i in range(N_CORES)])
    return out.astype(np.float32)


# revision 22
# speedup vs baseline: 2.2833x; 1.3512x over previous
"""BertAttention (B=8, S=1024, H=1024, 16 heads) on 8 TRN2 NeuronCores.

Strategy: data-parallel over batch -- core b computes batch element b
end-to-end (QKV proj, attention, output proj, residual, LayerNorm).
No collectives needed.

fp8 version: all big matmuls run in float8e4 (e4m3) with DoubleRow perf
mode (K=256 contraction per instruction, 2x PE throughput):
  - QKV projections: x, W in fp8; contraction over H = 4 DoubleRow steps.
  - Scores: q/k kept in bf16 (scores are PSUM-write-port-bound, dtype
    doesn't change cycles; bf16 improves accuracy for free).
  - PV: exp outputs written as fp8; DoubleRow pairs k-tiles; softmax
    denominators via a ones-column folded into V (row 64 of each head's
    65-wide V block).
  - Output projection: ctx in fp8, DoubleRow over ctx-tile pairs.
Engine budget per core: PE ~328k cycles (~137us), ACT exp ~133us
(overlapped), DVE does all PSUM->SBUF converts (GPSIMD cannot touch
PSUM), Pool does broadcasts + LN apply.

Emission order pipelines K[0],Q[0] first so ACT exp starts ~7us in, and
V-projection tiles are woven between per-head score blocks to avoid ACT
bubbles (scores PSUM double-buffering caps ACT run-ahead at 2 tiles).
"""

import sys

sys.path.insert(0, "/opt/trn_rl_repo")

import numpy as np

B, S, H = 8, 1024, 1024
NH, HD = 16, 64
LN_EPS = 1e-12
N_CORES = 8

MM_DTYPE = "fp8"  # "fp8" | "f32r"

_compiled = {}

NT = 8    # 128-row tiles per 1024 dim
NCH = 2   # 512-col chunks per 1024 dim
CH = 512
NKP = 4   # k-tile pairs (DoubleRow)


def _build_fp8(use_gb=True, n_reps=1):
    import concourse.tile as tile
    from concourse import bacc, mybir

    F32 = mybir.dt.float32
    FP8 = mybir.dt.float8e4
    BF16 = mybir.dt.bfloat16
    AF = mybir.ActivationFunctionType
    ALU = mybir.AluOpType
    DR = mybir.MatmulPerfMode.DoubleRow

    nc = bacc.Bacc("TRN2", target_bir_lowering=False)

    # swizzled layouts: t[p, kt*1024 + j] = M[kt*128 + p, j]
    xt_d = nc.dram_tensor("xt", [128, 8 * S], FP8, kind="ExternalInput")
    wq_d = nc.dram_tensor("wq", [128, 8 * H], FP8, kind="ExternalInput")
    wk_d = nc.dram_tensor("wk", [128, 8 * H], FP8, kind="ExternalInput")
    wv_d = nc.dram_tensor("wv", [128, 8 * H], FP8, kind="ExternalInput")
    wo_d = nc.dram_tensor("wo", [128, 8 * H], FP8, kind="ExternalInput")
    xr_d = nc.dram_tensor("xr", [S, H], F32, kind="ExternalInput")
    bq_d = nc.dram_tensor("bq", [128, 8], F32, kind="ExternalInput")
    bk_d = nc.dram_tensor("bk", [128, 8], F32, kind="ExternalInput")
    bv_d = nc.dram_tensor("bv", [1, H], F32, kind="ExternalInput")
    mask_d = nc.dram_tensor("mask", [128, 8], F32, kind="ExternalInput")
    gamma_d = nc.dram_tensor("gamma", [1, H], F32, kind="ExternalInput")
    beta_d = nc.dram_tensor("beta", [1, H], F32, kind="ExternalInput")
    out_d = nc.dram_tensor("out", [S, H], F32, kind="ExternalOutput")

    with tile.TileContext(nc) as tc:
      for _rep in range(n_reps):
        with (
            tc.tile_pool(name="consts", bufs=1) as cp,
            tc.tile_pool(name="wb", bufs=1) as wb,          # x + weights fp8
            tc.tile_pool(name="qk", bufs=1) as qkp,         # q/k bf16 tiles
            tc.tile_pool(name="vt", bufs=1) as vtp,         # v fp8 pair tiles
            tc.tile_pool(name="ctx", bufs=1) as ctxp_pool,  # ctx fp8 pair tiles
            tc.tile_pool(name="ep", bufs=8) as ep,          # exp fp8 pair tiles
            tc.tile_pool(name="rp", bufs=4) as rp,          # recip rows
            tc.tile_pool(name="rbp", bufs=4) as rbp,        # bcast recip
            tc.tile_pool(name="xrp", bufs=1) as xrp,        # residual tiles
            tc.tile_pool(name="ob", bufs=4) as obp,         # out f32 staging
            tc.tile_pool(name="st", bufs=4) as stp,         # LN stats
            tc.tile_pool(name="pp", bufs=2, space="PSUM") as pp,     # 2 banks
            tc.tile_pool(name="scps", bufs=2, space="PSUM") as scps, # 4 banks
            tc.tile_pool(name="cxps", bufs=2, space="PSUM") as cxps, # 2 banks
        ):
            # ---------------- DMAs: inputs ----------------
            xt8 = wb.tile([128, 8 * S], FP8, name="xt8")
            wq8 = wb.tile([128, 8 * H], FP8, name="wq8")
            wk8 = wb.tile([128, 8 * H], FP8, name="wk8")
            wv8 = wb.tile([128, 8 * H], FP8, name="wv8")
            wo8 = wb.tile([128, 8 * H], FP8, name="wo8")
            # split the two head-critical tensors across all three DMA
            # queues so K[0] projection can start ~2us earlier
            nc.sync.dma_start(out=xt8[0:64, :], in_=xt_d[0:64, :])
            nc.scalar.dma_start(out=xt8[64:128, :], in_=xt_d[64:128, :])
            nc.gpsimd.dma_start(out=wk8[0:64, :], in_=wk_d[0:64, :])
            nc.sync.dma_start(out=wk8[64:128, :], in_=wk_d[64:128, :])
            nc.scalar.dma_start(out=wq8, in_=wq_d[:])
            nc.gpsimd.dma_start(out=wv8, in_=wv_d[:])

            bq_sb = cp.tile([128, 8], F32)
            bk_sb = cp.tile([128, 8], F32)
            mask_sb = cp.tile([128, 8], F32)
            nc.sync.dma_start(out=bq_sb, in_=bq_d[:])
            nc.sync.dma_start(out=bk_sb, in_=bk_d[:])
            nc.sync.dma_start(out=mask_sb, in_=mask_d[:])
            bv_row = cp.tile([1, H], F32)
            gamma_row = cp.tile([1, H], F32)
            beta_row = cp.tile([1, H], F32)
            nc.sync.dma_start(out=bv_row, in_=bv_d[:])
            nc.sync.dma_start(out=gamma_row, in_=gamma_d[:])
            nc.sync.dma_start(out=beta_row, in_=beta_d[:])
            bv_sb = cp.tile([128, H], F32)
            nc.gpsimd.partition_broadcast(bv_sb[:], bv_row[:])
            if use_gb:
                gamma_sb = cp.tile([128, H], F32)
                beta_sb = cp.tile([128, H], F32)
                nc.gpsimd.partition_broadcast(gamma_sb[:], gamma_row[:])
                nc.gpsimd.partition_broadcast(beta_sb[:], beta_row[:])
            eps_sb = cp.tile([128, 1], F32)
            nc.vector.memset(eps_sb[:], LN_EPS)

            # q/k bf16 [dm 128, s 1024] per head-pair tile
            qt = [qkp.tile([128, S], BF16, name=f"qt{m}") for m in range(NT)]
            kt = [qkp.tile([128, S], BF16, name=f"kt{m}") for m in range(NT)]
            # v fp8 pair tiles: vp[j][p, kk, g*65+d]; s = (2j+kk)*128+p
            vp = [vtp.tile([128, 2 * NH * 65], FP8, name=f"vp{j}")
                  for j in range(NKP)]
            # ctx fp8 pair tiles: ctxp[j][p, tt, q]; c = (2j+tt)*128+p
            ctx = [ctxp_pool.tile([128, 2 * S], FP8, name=f"ctx{j}")
                   for j in range(NKP)]
            # ones columns for softmax denominators
            for j in range(NKP):
                v4 = vp[j][:].rearrange("p (i g e) -> p i g e", i=2, e=65)
                nc.gpsimd.memset(v4[:, :, :, 64:65], 1.0)

            def proj_qk(m, w8, b_sb, dst):
                # out [m-block 128, s]: lhsT = w slice, rhs = xt slices
                w3 = w8[:].rearrange("p (t j) -> p t j", t=8)
                x3 = xt8[:].rearrange("p (t j) -> p t j", t=8)
                for n in range(NCH):
                    ps = pp.tile([128, CH], F32, tag="pp", name="pp_t")
                    for r in range(4):
                        nc.tensor.matmul(
                            ps[:],
                            lhsT=w3[:, 2 * r:2 * r + 2, m * 128:(m + 1) * 128],
                            rhs=x3[:, 2 * r:2 * r + 2, n * CH:(n + 1) * CH],
                            start=(r == 0), stop=(r == 3), perf_mode=DR,
                        )
                    nc.vector.tensor_scalar_add(
                        dst[m][:, n * CH:(n + 1) * CH], ps[:], b_sb[:, m:m + 1]
                    )

            def proj_v(mk):
                # out [s-block 128, dv]: lhsT = xt slice, rhs = wv slices
                w3 = wv8[:].rearrange("p (t j) -> p t j", t=8)
                x3 = xt8[:].rearrange("p (t j) -> p t j", t=8)
                j, kk = mk // 2, mk % 2
                v4 = vp[j][:].rearrange("p (i g e) -> p i g e", i=2, e=65)
                for n in range(NCH):
                    ps = pp.tile([128, CH], F32, tag="pp", name="pp_t")
                    for r in range(4):
                        nc.tensor.matmul(
                            ps[:],
                            lhsT=x3[:, 2 * r:2 * r + 2, mk * 128:(mk + 1) * 128],
                            rhs=w3[:, 2 * r:2 * r + 2, n * CH:(n + 1) * CH],
                            start=(r == 0), stop=(r == 3), perf_mode=DR,
                        )
                    nc.vector.tensor_add(
                        v4[:, kk, n * 8:(n + 1) * 8, 0:64],
                        ps[:].rearrange("p (g e) -> p g e", e=64),
                        bv_sb[:, n * CH:(n + 1) * CH]
                        .rearrange("p (g e) -> p g e", e=64),
                    )

            import os
            ablate = os.environ.get("ABLATE", "")

            def head_block(g, cx, fillers=()):
                # scores + exp + PV for head g; cx = [cx_n0, cx_n1] accumulators
                # fillers: per-kp callables emitted between exp and PV, used to
                # weave V/QK projections into the head without starving ACT.
                t, hh = g // 2, g % 2
                p0 = hh * 64

                def pv(kp, e3):
                    v4 = vp[kp][:].rearrange("p (i m) -> p i m", i=2)
                    for n in range(NCH):
                        nc.tensor.matmul(
                            cx[n][:],
                            lhsT=v4[:, :, g * 65:(g + 1) * 65],
                            rhs=e3[:, :, n * CH:(n + 1) * CH],
                            start=(kp == 0), stop=(kp == NKP - 1),
                            perf_mode=DR,
                        )

                pend = None  # PV runs one kp group behind scores, so a PV
                for kp in range(NKP):  # stall never blocks score production
                    e_t = ep.tile([128, 2 * S], FP8, tag="e", name="e_t")
                    e3 = e_t[:].rearrange("p (i q) -> p i q", i=2)
                    for kk in range(2):
                        k = 2 * kp + kk
                        sc = scps.tile([128, S], F32, tag="sc", name="sc_t")
                        for n in range(NCH):
                            if ablate == "sc" and n == 1:
                                continue
                            nc.tensor.matmul(
                                sc[:, n * CH:(n + 1) * CH],
                                lhsT=kt[t][p0:p0 + 64, k * 128:(k + 1) * 128],
                                rhs=qt[t][p0:p0 + 64, n * CH:(n + 1) * CH],
                                start=True, stop=True,
                            )
                        if ablate == "exp" and kk == 1:
                            continue
                        nc.scalar.activation(
                            e3[:, kk, :], sc[:], AF.Exp,
                            bias=mask_sb[:, k:k + 1], scale=0.125,
                        )
                    if kp < len(fillers) and fillers[kp] is not None:
                        fillers[kp]()
                    if pend is not None:
                        pv(*pend)
                    pend = (kp, e3)
                pv(*pend)

            def normalize(g, cx):
                t, hh = g // 2, g % 2
                j, tt = t // 2, t % 2
                c3 = ctx[j][:].rearrange("p (i q) -> p i q", i=2)
                for n in range(NCH):
                    recip = rp.tile([1, CH], F32, tag="recip", name="recip_t")
                    nc.vector.reciprocal(recip[:], cx[n][64:65, :])
                    rb = rbp.tile([64, CH], F32, tag="rb", name="rb_t")
                    nc.gpsimd.partition_broadcast(rb[:], recip[:])
                    nc.vector.tensor_mul(
                        c3[hh * 64:hh * 64 + 64, tt, n * CH:(n + 1) * CH],
                        cx[n][0:64, :], rb[:],
                    )

            # ---------------- pipelined emission ----------------
            # Emission order IS the per-engine execution order and the
            # semantic (trace) order: every tile must be written before a
            # later-emitted reader. K[0]/Q[0] first so ACT exp starts early;
            # V-proj pairs are woven INSIDE head 0 (each before the PV(kp)
            # that consumes them), K/Q tile production into later heads.
            proj_qk(0, wk8, bk_sb, kt)
            proj_qk(0, wq8, bq_sb, qt)

            cx_of = {}

            def attn_head(g, fillers=()):
                if g > 0:
                    normalize(g - 1, cx_of.pop(g - 1))
                cx = [cxps.tile([65, CH], F32, tag="cx", name="cx_t")
                      for _ in range(NCH)]
                cx_of[g] = cx
                head_block(g, cx, fillers)

            attn_head(0, fillers=(
                lambda: (proj_v(0), proj_v(1)),
                lambda: (proj_v(2), proj_v(3)),
                lambda: (proj_v(4), proj_v(5)),
                lambda: (proj_v(6), proj_v(7)),
            ))
            attn_head(1, fillers=(
                lambda: proj_qk(1, wk8, bk_sb, kt),
                lambda: proj_qk(1, wq8, bq_sb, qt),
            ))
            attn_head(2, fillers=(
                lambda: proj_qk(2, wk8, bk_sb, kt),
            ))
            attn_head(3, fillers=(
                lambda: proj_qk(2, wq8, bq_sb, qt),
            ))

            def dma_prefetch():
                # wo + xr prefetch now that input DMA queues are free
                nc.scalar.dma_start(out=wo8, in_=wo_d[:])
                for mq in range(NT):
                    xr_t = xrp.tile([128, H], F32, name=f"xr{mq}")
                    (nc.sync if mq % 2 == 0 else nc.gpsimd).dma_start(
                        out=xr_t, in_=xr_d[mq * 128:(mq + 1) * 128, :]
                    )
                    xr_tiles.append(xr_t)

            xr_tiles = []
            for t in range(2, NT):
                g0 = 2 * t
                f0 = [lambda tt=t: proj_qk(tt + 1, wk8, bk_sb, kt)] \
                    if t + 1 < NT else []
                f1 = [lambda tt=t: proj_qk(tt + 1, wq8, bq_sb, qt)] \
                    if t + 1 < NT else []
                if t == 2:
                    f0.append(dma_prefetch)
                attn_head(g0, fillers=tuple(f0))
                attn_head(g0 + 1, fillers=tuple(f1))
            normalize(15, cx_of.pop(15))

            # ---------------- output proj + residual + LayerNorm ----------------
            # Post-sqrt tail (recip/nmur/apply/DMA) runs one mq behind, so
            # DVE's heavy fused-add never queues behind the ACT sqrt
            # round-trip (in-order DVE queue).
            wo3 = wo8[:].rearrange("p (t j) -> p t j", t=8)
            pend_ln = []

            def ln_tail(mq, o_t, mu, std):
                rstd = stp.tile([128, 1], F32, tag="rstd", name="rstd_t")
                nc.vector.reciprocal(rstd[:], std[:])
                # (x - mu) * rstd = rstd*x + (-mu*rstd)
                nmur = stp.tile([128, 1], F32, tag="nmur", name="nmur_t")
                nc.vector.tensor_scalar(
                    out=nmur[:], in0=mu[:], scalar1=rstd[:], scalar2=-1.0,
                    op0=ALU.mult, op1=ALU.mult,
                )
                of_t = obp.tile([128, H], F32, tag="ob", name="of_t")
                nc.gpsimd.tensor_scalar(
                    out=of_t[:], in0=o_t[:], scalar1=rstd[:], scalar2=nmur[:],
                    op0=ALU.mult, op1=ALU.add,
                )
                if use_gb:
                    nc.gpsimd.tensor_mul(of_t[:], of_t[:], gamma_sb[:])
                    nc.gpsimd.tensor_add(of_t[:], of_t[:], beta_sb[:])
                (nc.sync if mq % 2 == 0 else nc.scalar).dma_start(
                    out=out_d[mq * 128:(mq + 1) * 128, :], in_=of_t
                )

            for mq in range(NT):
                o_t = obp.tile([128, H], F32, tag="ob", name="ob_t")
                # O accumulator reuses the (now idle) scores psum pool
                ps = scps.tile([128, S], F32, tag="sc", name="sc_t")
                for n in range(NCH):
                    for j in range(NKP):
                        c3 = ctx[j][:].rearrange("p (i q) -> p i q", i=2)
                        nc.tensor.matmul(
                            ps[:, n * CH:(n + 1) * CH],
                            lhsT=c3[:, :, mq * 128:(mq + 1) * 128],
                            rhs=wo3[:, 2 * j:2 * j + 2, n * CH:(n + 1) * CH],
                            start=(j == 0), stop=(j == NKP - 1), perf_mode=DR,
                        )
                nc.vector.tensor_add(o_t[:], ps[:], xr_tiles[mq][:])
                stats = stp.tile([128, 2, 6], F32, tag="stats", name="stats_t")
                for sg in range(2):
                    nc.vector.bn_stats(
                        stats[:, sg, :], o_t[:, sg * CH:(sg + 1) * CH]
                    )
                mv = stp.tile([128, 2], F32, tag="mv", name="mv_t")
                nc.vector.bn_aggr(mv[:], stats[:])
                mu = mv[:, 0:1]
                var = mv[:, 1:2]
                std = stp.tile([128, 1], F32, tag="std", name="std_t")
                nc.scalar.activation(std[:], var[:], AF.Sqrt, bias=eps_sb[:])
                if pend_ln:
                    ln_tail(*pend_ln.pop())
                pend_ln.append((mq, o_t, mu, std))
            ln_tail(*pend_ln.pop())

    nc.compile()
    return nc


def _host_prep_fp8(hidden_states, attention_mask, Wq, bq, Wk, bk, Wv, bv,
                   Wo, bo, ln_gamma, ln_beta):
    import ml_dtypes

    f32 = np.float32
    fp8 = ml_dtypes.float8_e4m3
    hs = np.ascontiguousarray(hidden_states, dtype=f32)

    def swz(mat):
        # [1024, J] -> [128, 8*J] with t[p, kt*J + j] = mat[kt*128 + p, j]
        m = np.asarray(mat, f32).reshape(8, 128, -1).transpose(1, 0, 2)
        return np.ascontiguousarray(m.reshape(128, -1).astype(fp8))

    wq8 = swz(np.asarray(Wq, f32).T)
    wk8 = swz(np.asarray(Wk, f32).T)
    wv8 = swz(np.asarray(Wv, f32).T)
    wo8 = swz(np.asarray(Wo, f32).T)
    bq_r = np.ascontiguousarray(np.asarray(bq, f32).reshape(8, 128).T)
    bk_r = np.ascontiguousarray(np.asarray(bk, f32).reshape(8, 128).T)
    bv_r = np.ascontiguousarray(np.asarray(bv, f32).reshape(1, H))
    gamma_r = np.ascontiguousarray(np.asarray(ln_gamma, f32).reshape(1, H))
    beta_r = np.ascontiguousarray(np.asarray(ln_beta, f32).reshape(1, H))
    bo_r = np.asarray(bo, f32)
    mask = np.asarray(attention_mask, f32).reshape(B, S)

    in_maps = []
    for b in range(B):
        xt8 = swz(hs[b].T)
        xr = np.ascontiguousarray(hs[b] + bo_r[None, :])
        mask_r = np.ascontiguousarray(mask[b].reshape(8, 128).T)
        in_maps.append({
            "xt": xt8, "xr": xr,
            "wq": wq8, "wk": wk8, "wv": wv8, "wo": wo8,
            "bq": bq_r, "bk": bk_r, "bv": bv_r,
            "mask": mask_r, "gamma": gamma_r, "beta": beta_r,
        })
    return in_maps


def _host_prep(mm_dtype, **kw):
    assert mm_dtype == "fp8"
    return _host_prep_fp8(**kw)


def get_nc(mm_dtype=MM_DTYPE, n_reps=1, use_gb=True):
    key = (mm_dtype, n_reps, use_gb)
    if key not in _compiled:
        assert mm_dtype == "fp8"
        _compiled[key] = _build_fp8(use_gb, n_reps)
    return _compiled[key]


def kernel(hidden_states, attention_mask, Wq, bq, Wk, bk, Wv, bv, Wo, bo,
           ln_gamma, ln_beta):
    from concourse.bass_utils import run_bass_kernel_spmd

    use_gb = not (
        np.all(np.asarray(ln_gamma) == 1.0) and np.all(np.asarray(ln_beta) == 0.0)
    )
    nc = get_nc(MM_DTYPE, use_gb=use_gb)
    in_maps = _host_prep(MM_DTYPE, hidden_states=hidden_states,
                         attention_mask=attention_mask, Wq=Wq, bq=bq, Wk=Wk,
                         bk=bk, Wv=Wv, bv=bv, Wo=Wo, bo=bo,
                         ln_gamma=ln_gamma, ln_beta=ln_beta)
    res = run_bass_kernel_spmd(nc, in_maps, list(range(N_CORES)))
    out = np.stack([np.asarray(res.results[i]["out"]) for i in range(N_CORES)])
    return out.astype(np.float32)


# revision 23
# speedup vs baseline: 4.3307x; 1.8967x over previous
"""BertAttention (B=8, S=1024, H=1024, 16 heads) on 8 TRN2 NeuronCores.

Strategy: data-parallel over batch -- core b computes batch element b
end-to-end (QKV proj, attention, output proj, residual, LayerNorm).
No collectives needed.

fp8 version: all big matmuls run in float8e4 (e4m3) with DoubleRow perf
mode (K=256 contraction per instruction, 2x PE throughput):
  - QKV projections: x, W in fp8; contraction over H = 4 DoubleRow steps.
  - Scores: q/k kept in bf16 (scores are PSUM-write-port-bound, dtype
    doesn't change cycles; bf16 improves accuracy for free).
  - PV: exp outputs written as fp8; DoubleRow pairs k-tiles; softmax
    denominators via a ones-column folded into V (row 64 of each head's
    65-wide V block).
  - Output projection: ctx in fp8, DoubleRow over ctx-tile pairs.
Engine budget per core: PE ~328k cycles (~137us), ACT exp ~133us
(overlapped), DVE does all PSUM->SBUF converts (GPSIMD cannot touch
PSUM), Pool does broadcasts + LN apply.

Emission order pipelines K[0],Q[0] first so ACT exp starts ~7us in, and
V-projection tiles are woven between per-head score blocks to avoid ACT
bubbles (scores PSUM double-buffering caps ACT run-ahead at 2 tiles).
"""

import sys

sys.path.insert(0, "/opt/trn_rl_repo")

import numpy as np

B, S, H = 8, 1024, 1024
NH, HD = 16, 64
LN_EPS = 1e-12
N_CORES = 8

MM_DTYPE = "fp8"  # "fp8" | "f32r"

_compiled = {}

NT = 8    # 128-row tiles per 1024 dim
NCH = 2   # 512-col chunks per 1024 dim
CH = 512
NKP = 4   # k-tile pairs (DoubleRow)


def _build_fp8(use_gb=True, n_reps=1):
    import concourse.tile as tile
    from concourse import bacc, mybir

    F32 = mybir.dt.float32
    FP8 = mybir.dt.float8e4
    BF16 = mybir.dt.bfloat16
    AF = mybir.ActivationFunctionType
    ALU = mybir.AluOpType
    DR = mybir.MatmulPerfMode.DoubleRow

    nc = bacc.Bacc("TRN2", target_bir_lowering=False)

    # swizzled layouts: t[p, kt*1024 + j] = M[kt*128 + p, j]
    xt_d = nc.dram_tensor("xt", [128, 8 * S], FP8, kind="ExternalInput")
    wq_d = nc.dram_tensor("wq", [128, 8 * H], FP8, kind="ExternalInput")
    wk_d = nc.dram_tensor("wk", [128, 8 * H], FP8, kind="ExternalInput")
    wv_d = nc.dram_tensor("wv", [128, 8 * H], FP8, kind="ExternalInput")
    wo_d = nc.dram_tensor("wo", [128, 8 * H], FP8, kind="ExternalInput")
    xr_d = nc.dram_tensor("xr", [S, H], F32, kind="ExternalInput")
    bq_d = nc.dram_tensor("bq", [128, 8], F32, kind="ExternalInput")
    bk_d = nc.dram_tensor("bk", [128, 8], F32, kind="ExternalInput")
    bv_d = nc.dram_tensor("bv", [1, H], F32, kind="ExternalInput")
    mask_d = nc.dram_tensor("mask", [128, 8], F32, kind="ExternalInput")
    gamma_d = nc.dram_tensor("gamma", [1, H], F32, kind="ExternalInput")
    beta_d = nc.dram_tensor("beta", [1, H], F32, kind="ExternalInput")
    out_d = nc.dram_tensor("out", [S, H], F32, kind="ExternalOutput")

    with tile.TileContext(nc) as tc:
      for _rep in range(n_reps):
        with (
            tc.tile_pool(name="consts", bufs=1) as cp,
            tc.tile_pool(name="wb", bufs=1) as wb,          # x + weights fp8
            tc.tile_pool(name="qk", bufs=1) as qkp,         # q/k bf16 tiles
            tc.tile_pool(name="vt", bufs=1) as vtp,         # v fp8 pair tiles
            tc.tile_pool(name="ctx", bufs=1) as ctxp_pool,  # ctx fp8 pair tiles
            tc.tile_pool(name="ep", bufs=8) as ep,          # exp fp8 pair tiles
            tc.tile_pool(name="rp", bufs=4) as rp,          # recip rows
            tc.tile_pool(name="rbp", bufs=4) as rbp,        # bcast recip
            tc.tile_pool(name="xrp", bufs=1) as xrp,        # residual tiles
            tc.tile_pool(name="ob", bufs=4) as obp,         # out f32 staging
            tc.tile_pool(name="st", bufs=4) as stp,         # LN stats
            tc.tile_pool(name="pp", bufs=2, space="PSUM") as pp,     # 2 banks
            tc.tile_pool(name="scps", bufs=2, space="PSUM") as scps, # 4 banks
            tc.tile_pool(name="cxps", bufs=2, space="PSUM") as cxps, # 2 banks
        ):
            # ---------------- DMAs: inputs ----------------
            xt8 = wb.tile([128, 8 * S], FP8, name="xt8")
            wq8 = wb.tile([128, 8 * H], FP8, name="wq8")
            wk8 = wb.tile([128, 8 * H], FP8, name="wk8")
            wv8 = wb.tile([128, 8 * H], FP8, name="wv8")
            wo8 = wb.tile([128, 8 * H], FP8, name="wo8")
            # split the two head-critical tensors across all three DMA
            # queues so K[0] projection can start ~2us earlier
            nc.sync.dma_start(out=xt8[0:64, :], in_=xt_d[0:64, :])
            nc.scalar.dma_start(out=xt8[64:128, :], in_=xt_d[64:128, :])
            nc.gpsimd.dma_start(out=wk8[0:64, :], in_=wk_d[0:64, :])
            nc.sync.dma_start(out=wk8[64:128, :], in_=wk_d[64:128, :])
            nc.scalar.dma_start(out=wq8, in_=wq_d[:])
            nc.gpsimd.dma_start(out=wv8, in_=wv_d[:])

            bq_sb = cp.tile([128, 8], F32)
            bk_sb = cp.tile([128, 8], F32)
            mask_sb = cp.tile([128, 8], F32)
            nc.sync.dma_start(out=bq_sb, in_=bq_d[:])
            nc.sync.dma_start(out=bk_sb, in_=bk_d[:])
            nc.sync.dma_start(out=mask_sb, in_=mask_d[:])
            bv_row = cp.tile([1, H], F32)
            gamma_row = cp.tile([1, H], F32)
            beta_row = cp.tile([1, H], F32)
            nc.sync.dma_start(out=bv_row, in_=bv_d[:])
            nc.sync.dma_start(out=gamma_row, in_=gamma_d[:])
            nc.sync.dma_start(out=beta_row, in_=beta_d[:])
            bv_sb = cp.tile([128, H], F32)
            nc.gpsimd.partition_broadcast(bv_sb[:], bv_row[:])
            if use_gb:
                gamma_sb = cp.tile([128, H], F32)
                beta_sb = cp.tile([128, H], F32)
                nc.gpsimd.partition_broadcast(gamma_sb[:], gamma_row[:])
                nc.gpsimd.partition_broadcast(beta_sb[:], beta_row[:])
            eps_sb = cp.tile([128, 1], F32)
            nc.vector.memset(eps_sb[:], LN_EPS)

            # q/k bf16 [dm 128, s 1024] per head-pair tile
            qt = [qkp.tile([128, S], BF16, name=f"qt{m}") for m in range(NT)]
            kt = [qkp.tile([128, S], BF16, name=f"kt{m}") for m in range(NT)]
            # v fp8 pair tiles: vp[j][p, kk, g*65+d]; s = (2j+kk)*128+p
            vp = [vtp.tile([128, 2 * NH * 65], FP8, name=f"vp{j}")
                  for j in range(NKP)]
            # ctx fp8 pair tiles: ctxp[j][p, tt, q]; c = (2j+tt)*128+p
            ctx = [ctxp_pool.tile([128, 2 * S], FP8, name=f"ctx{j}")
                   for j in range(NKP)]
            # ones columns for softmax denominators
            for j in range(NKP):
                v4 = vp[j][:].rearrange("p (i g e) -> p i g e", i=2, e=65)
                nc.gpsimd.memset(v4[:, :, :, 64:65], 1.0)

            def proj_qk(m, w8, b_sb, dst):
                # out [m-block 128, s]: lhsT = w slice, rhs = xt slices
                w3 = w8[:].rearrange("p (t j) -> p t j", t=8)
                x3 = xt8[:].rearrange("p (t j) -> p t j", t=8)
                for n in range(NCH):
                    ps = pp.tile([128, CH], F32, tag="pp", name="pp_t")
                    for r in range(4):
                        nc.tensor.matmul(
                            ps[:],
                            lhsT=w3[:, 2 * r:2 * r + 2, m * 128:(m + 1) * 128],
                            rhs=x3[:, 2 * r:2 * r + 2, n * CH:(n + 1) * CH],
                            start=(r == 0), stop=(r == 3), perf_mode=DR,
                        )
                    nc.vector.tensor_scalar_add(
                        dst[m][:, n * CH:(n + 1) * CH], ps[:], b_sb[:, m:m + 1]
                    )

            def proj_v(mk):
                # out [s-block 128, dv]: lhsT = xt slice, rhs = wv slices
                w3 = wv8[:].rearrange("p (t j) -> p t j", t=8)
                x3 = xt8[:].rearrange("p (t j) -> p t j", t=8)
                j, kk = mk // 2, mk % 2
                v4 = vp[j][:].rearrange("p (i g e) -> p i g e", i=2, e=65)
                for n in range(NCH):
                    ps = pp.tile([128, CH], F32, tag="pp", name="pp_t")
                    for r in range(4):
                        nc.tensor.matmul(
                            ps[:],
                            lhsT=x3[:, 2 * r:2 * r + 2, mk * 128:(mk + 1) * 128],
                            rhs=w3[:, 2 * r:2 * r + 2, n * CH:(n + 1) * CH],
                            start=(r == 0), stop=(r == 3), perf_mode=DR,
                        )
                    nc.vector.tensor_add(
                        v4[:, kk, n * 8:(n + 1) * 8, 0:64],
                        ps[:].rearrange("p (g e) -> p g e", e=64),
                        bv_sb[:, n * CH:(n + 1) * CH]
                        .rearrange("p (g e) -> p g e", e=64),
                    )

            import os
            ablate = os.environ.get("ABLATE", "")

            def head_block(g, cx, fillers=()):
                # scores + exp + PV for head g; cx = [cx_n0, cx_n1] accumulators
                # fillers: per-kp callables emitted between exp and PV, used to
                # weave V/QK projections into the head without starving ACT.
                t, hh = g // 2, g % 2
                p0 = hh * 64

                def pv(kp, e3):
                    v4 = vp[kp][:].rearrange("p (i m) -> p i m", i=2)
                    for n in range(NCH):
                        nc.tensor.matmul(
                            cx[n][:],
                            lhsT=v4[:, :, g * 65:(g + 1) * 65],
                            rhs=e3[:, :, n * CH:(n + 1) * CH],
                            start=(kp == 0), stop=(kp == NKP - 1),
                            perf_mode=DR,
                        )

                pend = None  # PV runs one kp group behind scores, so a PV
                for kp in range(NKP):  # stall never blocks score production
                    e_t = ep.tile([128, 2 * S], FP8, tag="e", name="e_t")
                    e3 = e_t[:].rearrange("p (i q) -> p i q", i=2)
                    for kk in range(2):
                        k = 2 * kp + kk
                        sc = scps.tile([128, S], F32, tag="sc", name="sc_t")
                        for n in range(NCH):
                            if ablate == "sc" and n == 1:
                                continue
                            nc.tensor.matmul(
                                sc[:, n * CH:(n + 1) * CH],
                                lhsT=kt[t][p0:p0 + 64, k * 128:(k + 1) * 128],
                                rhs=qt[t][p0:p0 + 64, n * CH:(n + 1) * CH],
                                start=True, stop=True,
                            )
                        if ablate == "exp" and kk == 1:
                            continue
                        nc.scalar.activation(
                            e3[:, kk, :], sc[:], AF.Exp,
                            bias=mask_sb[:, k:k + 1], scale=0.125,
                        )
                    if kp < len(fillers) and fillers[kp] is not None:
                        fillers[kp]()
                    if pend is not None:
                        pv(*pend)
                    pend = (kp, e3)
                pv(*pend)

            def normalize(g, cx):
                t, hh = g // 2, g % 2
                j, tt = t // 2, t % 2
                c3 = ctx[j][:].rearrange("p (i q) -> p i q", i=2)
                for n in range(NCH):
                    recip = rp.tile([1, CH], F32, tag="recip", name="recip_t")
                    nc.vector.reciprocal(recip[:], cx[n][64:65, :])
                    rb = rbp.tile([64, CH], F32, tag="rb", name="rb_t")
                    nc.gpsimd.partition_broadcast(rb[:], recip[:])
                    nc.vector.tensor_mul(
                        c3[hh * 64:hh * 64 + 64, tt, n * CH:(n + 1) * CH],
                        cx[n][0:64, :], rb[:],
                    )

            # ---------------- pipelined emission ----------------
            # Emission order IS the per-engine execution order and the
            # semantic (trace) order: every tile must be written before a
            # later-emitted reader. K[0]/Q[0] first so ACT exp starts early;
            # V-proj pairs are woven INSIDE head 0 (each before the PV(kp)
            # that consumes them), K/Q tile production into later heads.
            proj_qk(0, wk8, bk_sb, kt)
            proj_qk(0, wq8, bq_sb, qt)

            cx_of = {}

            def attn_head(g, fillers=()):
                if g > 0:
                    normalize(g - 1, cx_of.pop(g - 1))
                cx = [cxps.tile([65, CH], F32, tag="cx", name="cx_t")
                      for _ in range(NCH)]
                cx_of[g] = cx
                head_block(g, cx, fillers)

            attn_head(0, fillers=(
                lambda: (proj_v(0), proj_v(1)),
                lambda: (proj_v(2), proj_v(3)),
                lambda: (proj_v(4), proj_v(5)),
                lambda: (proj_v(6), proj_v(7)),
            ))
            attn_head(1, fillers=(
                lambda: proj_qk(1, wk8, bk_sb, kt),
                lambda: proj_qk(1, wq8, bq_sb, qt),
            ))
            attn_head(2, fillers=(
                lambda: proj_qk(2, wk8, bk_sb, kt),
            ))
            attn_head(3, fillers=(
                lambda: proj_qk(2, wq8, bq_sb, qt),
            ))

            def dma_prefetch():
                # wo + xr prefetch now that input DMA queues are free
                nc.scalar.dma_start(out=wo8, in_=wo_d[:])
                for mq in range(NT):
                    xr_t = xrp.tile([128, H], F32, name=f"xr{mq}")
                    (nc.sync if mq % 2 == 0 else nc.gpsimd).dma_start(
                        out=xr_t, in_=xr_d[mq * 128:(mq + 1) * 128, :]
                    )
                    xr_tiles.append(xr_t)

            xr_tiles = []
            for t in range(2, NT):
                g0 = 2 * t
                f0 = [lambda tt=t: proj_qk(tt + 1, wk8, bk_sb, kt)] \
                    if t + 1 < NT else []
                f1 = [lambda tt=t: proj_qk(tt + 1, wq8, bq_sb, qt)] \
                    if t + 1 < NT else []
                if t == 2:
                    f0.append(dma_prefetch)
                attn_head(g0, fillers=tuple(f0))
                attn_head(g0 + 1, fillers=tuple(f1))
            normalize(15, cx_of.pop(15))

            # ---------------- output proj + residual + LayerNorm ----------------
            # Post-sqrt tail (recip/nmur/apply/DMA) runs one mq behind, so
            # DVE's heavy fused-add never queues behind the ACT sqrt
            # round-trip (in-order DVE queue).
            wo3 = wo8[:].rearrange("p (t j) -> p t j", t=8)
            pend_ln = []

            def ln_tail(mq, o_t, mu, std):
                rstd = stp.tile([128, 1], F32, tag="rstd", name="rstd_t")
                nc.vector.reciprocal(rstd[:], std[:])
                # (x - mu) * rstd = rstd*x + (-mu*rstd)
                nmur = stp.tile([128, 1], F32, tag="nmur", name="nmur_t")
                nc.vector.tensor_scalar(
                    out=nmur[:], in0=mu[:], scalar1=rstd[:], scalar2=-1.0,
                    op0=ALU.mult, op1=ALU.mult,
                )
                of_t = obp.tile([128, H], F32, tag="ob", name="of_t")
                nc.gpsimd.tensor_scalar(
                    out=of_t[:], in0=o_t[:], scalar1=rstd[:], scalar2=nmur[:],
                    op0=ALU.mult, op1=ALU.add,
                )
                if use_gb:
                    nc.gpsimd.tensor_mul(of_t[:], of_t[:], gamma_sb[:])
                    nc.gpsimd.tensor_add(of_t[:], of_t[:], beta_sb[:])
                nc.sync.dma_start(
                    out=out_d[mq * 128:(mq + 1) * 128, :], in_=of_t
                )

            for mq in range(NT):
                o_t = obp.tile([128, H], F32, tag="ob", name="ob_t")
                # O accumulator reuses the (now idle) scores psum pool
                ps = scps.tile([128, S], F32, tag="sc", name="sc_t")
                for n in range(NCH):
                    for j in range(NKP):
                        c3 = ctx[j][:].rearrange("p (i q) -> p i q", i=2)
                        nc.tensor.matmul(
                            ps[:, n * CH:(n + 1) * CH],
                            lhsT=c3[:, :, mq * 128:(mq + 1) * 128],
                            rhs=wo3[:, 2 * j:2 * j + 2, n * CH:(n + 1) * CH],
                            start=(j == 0), stop=(j == NKP - 1), perf_mode=DR,
                        )
                # residual add fused with row-sum on DVE; x^2 row-sum on the
                # (otherwise idle) ACT engine; LN stats from the two sums
                sumx = stp.tile([128, 1], F32, tag="sumx", name="sumx_t")
                nc.vector.scalar_tensor_tensor(
                    out=o_t[:], in0=ps[:], scalar=0.0, in1=xr_tiles[mq][:],
                    op0=ALU.add, op1=ALU.add, accum_out=sumx[:],
                )
                sq_t = obp.tile([128, H], F32, tag="sq", name="sq_t")
                sumsq = stp.tile([128, 1], F32, tag="sumsq", name="sumsq_t")
                nc.scalar.activation(
                    sq_t[:], o_t[:], AF.Square, accum_out=sumsq[:],
                )
                mu = stp.tile([128, 1], F32, tag="mu", name="mu_t")
                nc.vector.tensor_scalar_mul(mu[:], sumx[:], 1.0 / H)
                musq = stp.tile([128, 1], F32, tag="musq", name="musq_t")
                nc.vector.tensor_scalar(
                    out=musq[:], in0=mu[:], scalar1=mu[:], scalar2=None,
                    op0=ALU.mult,
                )
                var = stp.tile([128, 1], F32, tag="var", name="var_t")
                nc.vector.tensor_scalar(
                    out=var[:], in0=sumsq[:], scalar1=1.0 / H, scalar2=musq[:],
                    op0=ALU.mult, op1=ALU.subtract,
                )
                std = stp.tile([128, 1], F32, tag="std", name="std_t")
                nc.scalar.activation(std[:], var[:], AF.Sqrt, bias=eps_sb[:])
                if pend_ln:
                    ln_tail(*pend_ln.pop())
                pend_ln.append((mq, o_t, mu, std))
            ln_tail(*pend_ln.pop())

    nc.compile()
    return nc


def _host_prep_fp8(hidden_states, attention_mask, Wq, bq, Wk, bk, Wv, bv,
                   Wo, bo, ln_gamma, ln_beta):
    import ml_dtypes

    f32 = np.float32
    fp8 = ml_dtypes.float8_e4m3
    hs = np.ascontiguousarray(hidden_states, dtype=f32)

    def swz(mat):
        # [1024, J] -> [128, 8*J] with t[p, kt*J + j] = mat[kt*128 + p, j]
        m = np.asarray(mat, f32).reshape(8, 128, -1).transpose(1, 0, 2)
        return np.ascontiguousarray(m.reshape(128, -1).astype(fp8))

    wq8 = swz(np.asarray(Wq, f32).T)
    wk8 = swz(np.asarray(Wk, f32).T)
    wv8 = swz(np.asarray(Wv, f32).T)
    wo8 = swz(np.asarray(Wo, f32).T)
    bq_r = np.ascontiguousarray(np.asarray(bq, f32).reshape(8, 128).T)
    bk_r = np.ascontiguousarray(np.asarray(bk, f32).reshape(8, 128).T)
    bv_r = np.ascontiguousarray(np.asarray(bv, f32).reshape(1, H))
    gamma_r = np.ascontiguousarray(np.asarray(ln_gamma, f32).reshape(1, H))
    beta_r = np.ascontiguousarray(np.asarray(ln_beta, f32).reshape(1, H))
    bo_r = np.asarray(bo, f32)
    mask = np.asarray(attention_mask, f32).reshape(B, S)

    in_maps = []
    for b in range(B):
        xt8 = swz(hs[b].T)
        xr = np.ascontiguousarray(hs[b] + bo_r[None, :])
        mask_r = np.ascontiguousarray(mask[b].reshape(8, 128).T)
        in_maps.append({
            "xt": xt8, "xr": xr,
            "wq": wq8, "wk": wk8, "wv": wv8, "wo": wo8,
            "bq": bq_r, "bk": bk_r, "bv": bv_r,
            "mask": mask_r, "gamma": gamma_r, "beta": beta_r,
        })
    return in_maps


def _host_prep(mm_dtype, **kw):
    assert mm_dtype == "fp8"
    return _host_prep_fp8(**kw)


def get_nc(mm_dtype=MM_DTYPE, n_reps=1, use_gb=True):
    key = (mm_dtype, n_reps, use_gb)
    if key not in _compiled:
        assert mm_dtype == "fp8"
        _compiled[key] = _build_fp8(use_gb, n_reps)
    return _compiled[key]


def kernel(hidden_states, attention_mask, Wq, bq, Wk, bk, Wv, bv, Wo, bo,
           ln_gamma, ln_beta):
    from concourse.bass_utils import run_bass_kernel_spmd

    use_gb = not (
        np.all(np.asarray(ln_gamma) == 1.0) and np.all(np.asarray(ln_beta) == 0.0)
    )
    nc = get_nc(MM_DTYPE, use_gb=use_gb)
    in_maps = _host_prep(MM_DTYPE, hidden_states=hidden_states,
                         attention_mask=attention_mask, Wq=Wq, bq=bq, Wk=Wk,
                         bk=bk, Wv=Wv, bv=bv, Wo=Wo, bo=bo,
                         ln_gamma=ln_gamma, ln_beta=ln_beta)
    res = run_bass_kernel_spmd(nc, in_maps, list(range(N_CORES)))
    out = np.stack([np.asarray(res.results[i]["out"]) for i in range(N_CORES)])
    return out.astype(np.float32)
